# revision 6
# baseline (speedup 1.0000x reference)
"""Multi-head attention (B=2, S=2048, D=1024, H=16) on 8 NeuronCores.

Sharding: tensor-parallel over heads — 2 heads per core. Each core computes
q/k/v projections for its 128 output columns, full attention for its 2 heads
(both batches), and a partial out-projection [4096, 1024] in bf16. Host sums
the 8 partials and adds the output bias.

Schedule (single fused pipeline, ACT-exp is the long pole):
  - Q^T/K^T/V projections and deferred out-projections are emitted as
    "filler" PE work interleaved into the attention kt-loop, which is paced
    by the ACT engine's exp throughput. Attention for batch 0 starts as soon
    as Q(b0,qg0) + K(b0) are projected; everything else fills PE idle slots.
  - scores for the two heads are issued back-to-back with separate PSUM
    tiles; head0 occupies PE rows 0-63, head1 rows 64-127 (tile_position
    auto-derived), so the two matmuls co-execute on disjoint row groups.
  - V carries an extra all-ones column so attn@[V|1] yields the softmax
    denominator (row 64) along with the unnormalized output (rows 0..63).
  - softmax skips max-subtraction: scores are ~N(0, 0.33^2) by construction.
  - denominators: DVE reciprocal_approx_fast -> GpSimd partition_broadcast
    -> DVE multiply during PSUM evacuation.
  - weights / xT are host-permuted so every DMA is 2KB-contiguous per
    partition.
"""

import collections
import os

import ml_dtypes
import numpy as np

B, S, D, H = 2, 2048, 1024, 16
HD = D // H          # 64
BS = B * S           # 4096 tokens
NCORES = 8
HPC = H // NCORES    # heads per core = 2
CPC = HPC * HD       # output cols per core = 128
KC = D // 128        # contract chunks = 8
QCH = 512            # matmul moving free dim
NKT = S // 128       # 16 key tiles per batch
QG = 1024            # q-group width
NQG = S // QG        # 2 q-groups per batch

BF16 = ml_dtypes.bfloat16

_prog = None


def _build_program():
    import concourse.bacc as bacc
    import concourse.tile as tile
    from concourse import mybir

    f32 = mybir.dt.float32
    bf16 = mybir.dt.bfloat16
    AF = mybir.ActivationFunctionType

    nc = bacc.Bacc("TRN2", debug=False, enable_asserts=False, num_devices=NCORES)

    xT = nc.dram_tensor("xT", [128, KC, BS], bf16, kind="ExternalInput").ap()
    wq = nc.dram_tensor("wq", [128, KC, CPC], bf16, kind="ExternalInput").ap()
    wk = nc.dram_tensor("wk", [128, KC, CPC], bf16, kind="ExternalInput").ap()
    wv = nc.dram_tensor("wv", [128, KC, CPC], bf16, kind="ExternalInput").ap()
    wo = nc.dram_tensor("wo", [CPC, D], bf16, kind="ExternalInput").ap()
    bq = nc.dram_tensor("bq", [CPC, 1], f32, kind="ExternalInput").ap()
    bk = nc.dram_tensor("bk", [CPC, 1], f32, kind="ExternalInput").ap()
    bv = nc.dram_tensor("bv", [1, CPC], bf16, kind="ExternalInput").ap()
    out = nc.dram_tensor("out", [BS, D], bf16, kind="ExternalOutput").ap()

    SCALE = float(1.0 / np.sqrt(HD))

    with tile.TileContext(nc) as tc:
        with (
            tc.tile_pool(name="big", bufs=1) as big,
            tc.tile_pool(name="sm", bufs=1) as sm,
            tc.tile_pool(name="attn", bufs=2) as attn,
            tc.tile_pool(name="etp", bufs=2) as etp,
            tc.tile_pool(name="ostage", bufs=2) as ostage,
            tc.tile_pool(name="ps", bufs=2, space="PSUM") as ps,
        ):
            # ---- resident SBUF tensors ----
            xt_sb = big.tile([128, KC, BS], bf16, name="xt_sb", tag="xt")
            qt_sb = big.tile([128, BS], bf16, name="qt_sb", tag="qt")
            kt_sb = big.tile([128, BS], bf16, name="kt_sb", tag="kt")
            # V|ones per head: [keys(128), keytile(32), head(2), 64 V + 1 ones]
            v_sb = big.tile([128, B * NKT, HPC, HD + 1], bf16, name="v_sb", tag="v")
            wo_sb = big.tile([128, D], bf16, name="wo_sb", tag="wo")

            wq_sb = sm.tile([128, KC, CPC], bf16, name="wq_sb", tag="wq")
            wk_sb = sm.tile([128, KC, CPC], bf16, name="wk_sb", tag="wk")
            wv_sb = sm.tile([128, KC, CPC], bf16, name="wv_sb", tag="wv")
            bq_sb = sm.tile([CPC, 1], f32, name="bq_sb", tag="bq")
            bk_sb = sm.tile([CPC, 1], f32, name="bk_sb", tag="bk")
            bv_sb = sm.tile([1, CPC], bf16, name="bv_sb", tag="bv")
            ones_bf = sm.tile([1, 128], bf16, name="ones_bf", tag="onesb")

            # flat [128, kt*head, HD+1] view for memset / projection evacuation
            v3 = v_sb.rearrange("p k h c -> p (k h) c")
            nc.vector.memset(ones_bf, 1.0)
            nc.vector.memset(v3[:, :, HD : HD + 1], 1.0)

            # ---- DMAs, ordered so the first projection can start ASAP ----
            nc.sync.dma_start(out=wq_sb, in_=wq)
            for c in range(KC):
                nc.sync.dma_start(out=xt_sb[:, c, 0:QG], in_=xT[:, c, 0:QG])
            nc.sync.dma_start(out=wk_sb, in_=wk)
            nc.sync.dma_start(out=xt_sb[:, :, QG : 2 * QG], in_=xT[:, :, QG : 2 * QG])
            nc.sync.dma_start(out=wv_sb, in_=wv)
            nc.sync.dma_start(out=bq_sb, in_=bq)
            nc.sync.dma_start(out=bk_sb, in_=bk)
            nc.sync.dma_start(out=bv_sb, in_=bv)
            nc.sync.dma_start(out=wo_sb, in_=wo)
            nc.sync.dma_start(out=xt_sb[:, :, 2 * QG : 3 * QG], in_=xT[:, :, 2 * QG : 3 * QG])
            nc.sync.dma_start(out=xt_sb[:, :, 3 * QG : 4 * QG], in_=xT[:, :, 3 * QG : 4 * QG])

            # ---- filler machinery: PE work interleaved into attention ----
            filler_q = collections.deque()

            def drain(n):
                done = 0
                while filler_q and done < n:
                    try:
                        next(filler_q[0])
                        done += 1
                    except StopIteration:
                        filler_q.popleft()

            def gen_qkproj(dst, w_sb, b_sb, tb, nm):
                """One 1024-token block of a Q^T/K^T projection (2 chunks)."""
                pp = ps.tile([128, QG], f32, name=f"pp_{nm}", tag="sp")
                for qh in range(2):
                    for c in range(KC):
                        nc.tensor.matmul(
                            pp[:, qh * QCH : (qh + 1) * QCH],
                            lhsT=w_sb[:, c, :],
                            rhs=xt_sb[:, c, tb * QG + qh * QCH : tb * QG + (qh + 1) * QCH],
                            start=(c == 0),
                            stop=(c == KC - 1),
                        )
                    yield
                nc.vector.tensor_scalar_add(dst[:, tb * QG : (tb + 1) * QG], pp, b_sb)

            def gen_vproj(b):
                """V projection (+bias) for batch b, natural [keys, cols]."""
                for half in range(2):
                    vp = ps.tile([128, QG], f32, name=f"vp{b}{half}", tag="sp")
                    for k8 in range(8):
                        kt = b * NKT + half * 8 + k8
                        sl = slice(k8 * 128, (k8 + 1) * 128)
                        for c in range(KC):
                            nc.tensor.matmul(
                                vp[:, sl],
                                lhsT=xt_sb[:, c, kt * 128 : (kt + 1) * 128],
                                rhs=wv_sb[:, c, :],
                                start=(c == 0),
                                stop=False,
                            )
                        nc.tensor.matmul(vp[:, sl], lhsT=ones_bf, rhs=bv_sb, start=False, stop=True)
                        nc.vector.tensor_copy(
                            v3[:, kt * HPC : (kt + 1) * HPC, 0:HD],
                            vp[:, sl].rearrange("p (h c) -> p h c", c=HD),
                        )
                        yield

            def gen_outproj(b, qg, ot, tail=False):
                """Partial out-projection for one q-group (8 q-tiles)."""
                for j, qt in enumerate(range(qg * 8, (qg + 1) * 8)):
                    pq = ps.tile([128, QG], f32, name=f"pq{b}{qt}", tag="sp")
                    for nh in range(2):
                        nc.tensor.matmul(
                            pq[:, nh * QCH : (nh + 1) * QCH],
                            lhsT=ot[:, qt * 128 : (qt + 1) * 128],
                            rhs=wo_sb[:, nh * QCH : (nh + 1) * QCH],
                            start=True,
                            stop=True,
                        )
                    os_ = ostage.tile([128, QG], bf16, name=f"os{b}{qt}", tag="os", bufs=3)
                    if tail and j % 2 == 0:
                        nc.scalar.copy(os_, pq)
                    else:
                        nc.vector.tensor_copy(os_, pq)
                    nc.sync.dma_start(
                        out=out[b * S + qt * 128 : b * S + (qt + 1) * 128, :], in_=os_
                    )
                    yield

            # ---- prefix: minimum projections before attention(b0, qg0) ----
            for g in (
                gen_qkproj(qt_sb, wq_sb, bq_sb, 0, "q0"),
                gen_qkproj(kt_sb, wk_sb, bk_sb, 0, "k0"),
                gen_qkproj(kt_sb, wk_sb, bk_sb, 1, "k1"),
            ):
                for _ in g:
                    pass
            vgen0 = gen_vproj(0)
            for _ in range(4):
                next(vgen0)
            filler_q.append(vgen0)
            filler_q.append(gen_qkproj(qt_sb, wq_sb, bq_sb, 1, "q1"))
            filler_q.append(gen_qkproj(kt_sb, wk_sb, bk_sb, 2, "k2"))
            filler_q.append(gen_qkproj(kt_sb, wk_sb, bk_sb, 3, "k3"))
            filler_q.append(gen_qkproj(qt_sb, wq_sb, bq_sb, 2, "q2"))
            filler_q.append(gen_vproj(1))
            filler_q.append(gen_qkproj(qt_sb, wq_sb, bq_sb, 3, "q3"))

            # ---- attention, ACT-paced; PE idle slots consumed by fillers ----
            ot_tiles = {}
            for b in range(B):
                ot_tiles[b] = attn.tile([128, S], bf16, name=f"ot{b}", tag="ot")
            for b in range(B):
                ot = ot_tiles[b]
                for qg in range(NQG):
                    q0 = b * S + qg * QG
                    et = [
                        etp.tile([128, NKT, QG], bf16, name=f"et{b}{qg}{h}", tag="et")
                        for h in range(HPC)
                    ]
                    op = [
                        ps.tile([HD + 1, QG], f32, name=f"op{b}{qg}{h}", tag="op")
                        for h in range(HPC)
                    ]
                    for kt in range(NKT):
                        sps = [
                            ps.tile([128, QG], f32, name=f"sp{b}{qg}{h}{kt}", tag="sp")
                            for h in range(HPC)
                        ]
                        # head0 rows 0-63, head1 rows 64-127: adjacent issue ->
                        # the pair co-executes on disjoint PE row groups
                        for qh in range(2):
                            for h in range(HPC):
                                hp = h * HD
                                nc.tensor.matmul(
                                    sps[h][:, qh * QCH : (qh + 1) * QCH],
                                    lhsT=kt_sb[hp : hp + HD, b * S + kt * 128 : b * S + (kt + 1) * 128],
                                    rhs=qt_sb[hp : hp + HD, q0 + qh * QCH : q0 + (qh + 1) * QCH],
                                    start=True,
                                    stop=True,
                                )
                        for h in range(HPC):
                            nc.scalar.activation(et[h][:, kt, :], sps[h], AF.Exp, scale=SCALE)
                        for h in range(HPC):
                            for qc in range(2):
                                nc.tensor.matmul(
                                    op[h][:, qc * QCH : (qc + 1) * QCH],
                                    lhsT=v_sb[:, b * NKT + kt, h, :],
                                    rhs=et[h][:, kt, qc * QCH : (qc + 1) * QCH],
                                    start=(kt == 0),
                                    stop=(kt == NKT - 1),
                                )
                        drain(2)
                    # normalize: recip (DVE) -> broadcast (GpSimd) -> mul (DVE)
                    rcs, rbss = [], []
                    for h in range(HPC):
                        rc = ostage.tile([1, QG], f32, name=f"rc{b}{qg}{h}", tag="rc", bufs=2)
                        nc.vector.reciprocal(rc, op[h][HD : HD + 1, :])
                        rcs.append(rc)
                    for h in range(HPC):
                        rbs = ostage.tile([HD, QG], f32, name=f"rbs{b}{qg}{h}", tag="rbs", bufs=2)
                        nc.gpsimd.partition_broadcast(rbs, rcs[h])
                        rbss.append(rbs)
                    for h in range(HPC):
                        hp = h * HD
                        nc.vector.tensor_mul(
                            ot[hp : hp + HD, qg * QG : (qg + 1) * QG],
                            op[h][0:HD, :],
                            rbss[h],
                        )
                    last = b == B - 1 and qg == NQG - 1
                    filler_q.append(gen_outproj(b, qg, ot, tail=last))
            drain(10000)

    nc.compile()
    return nc


def _get_prog():
    global _prog
    if _prog is None:
        _prog = _build_program()
    return _prog


def _perm_ckc(a):
    """[D, N] -> [128, KC, N] with partition-contiguous rows."""
    return np.ascontiguousarray(a.reshape(KC, 128, -1).transpose(1, 0, 2))


def kernel(x, Wq, bq, Wk, bk, Wv, bv, Wo, bo):
    from concourse import bass_utils

    nc = _get_prog()

    xT = np.asarray(x, dtype=np.float32).reshape(BS, D).T.astype(BF16)
    xTp = _perm_ckc(xT)

    in_maps = []
    for c in range(NCORES):
        cols = slice(c * CPC, (c + 1) * CPC)
        in_maps.append(
            {
                "xT": xTp,
                "wq": _perm_ckc(Wq[cols, :].T.astype(BF16)),
                "wk": _perm_ckc(Wk[cols, :].T.astype(BF16)),
                "wv": _perm_ckc(Wv[cols, :].T.astype(BF16)),
                "wo": np.ascontiguousarray(Wo[:, cols].T).astype(BF16),
                "bq": np.asarray(bq[cols], np.float32).reshape(CPC, 1),
                "bk": np.asarray(bk[cols], np.float32).reshape(CPC, 1),
                "bv": np.asarray(bv[cols], np.float32).reshape(1, CPC).astype(BF16),
            }
        )

    res = bass_utils.run_bass_kernel_spmd(
        nc,
        in_maps,
        core_ids=list(range(NCORES)),
        trace=bool(int(os.environ.get("KERNEL_TRACE", "0"))),
    )
    kernel.last_results = res

    acc = np.zeros((BS, D), np.float64)
    for c in range(NCORES):
        acc += res.results[c]["out"].astype(np.float64)
    acc += np.asarray(bo, np.float64)[None, :]
    return acc.reshape(B, S, D).astype(np.float32)


# revision 8
# speedup vs baseline: 1.0166x; 1.0166x over previous
"""Multi-head attention (B=2, S=2048, D=1024, H=16) on 8 NeuronCores.

Sharding: tensor-parallel over heads — 2 heads per core. Each core computes
q/k/v projections for its 128 output columns, full attention for its 2 heads
(both batches), and a partial out-projection [4096, 1024] in bf16. Host sums
the 8 partials and adds the output bias.

Schedule (single fused pipeline, ACT-exp is the long pole):
  - Q^T/K^T/V projections and deferred out-projections are emitted as
    "filler" PE work interleaved into the attention kt-loop, which is paced
    by the ACT engine's exp throughput. Attention for batch 0 starts as soon
    as Q(b0,qg0) + K(b0) are projected; everything else fills PE idle slots.
  - scores for the two heads are issued back-to-back with separate PSUM
    tiles; head0 occupies PE rows 0-63, head1 rows 64-127 (tile_position
    auto-derived), so the two matmuls co-execute on disjoint row groups.
  - V carries an extra all-ones column so attn@[V|1] yields the softmax
    denominator (row 64) along with the unnormalized output (rows 0..63).
  - softmax skips max-subtraction: scores are ~N(0, 0.33^2) by construction.
  - denominators: DVE reciprocal_approx_fast -> GpSimd partition_broadcast
    -> DVE multiply during PSUM evacuation.
  - weights / xT are host-permuted so every DMA is 2KB-contiguous per
    partition.
"""

import collections
import os

import ml_dtypes
import numpy as np

B, S, D, H = 2, 2048, 1024, 16
HD = D // H          # 64
BS = B * S           # 4096 tokens
NCORES = 8
HPC = H // NCORES    # heads per core = 2
CPC = HPC * HD       # output cols per core = 128
KC = D // 128        # contract chunks = 8
QCH = 512            # matmul moving free dim
NKT = S // 128       # 16 key tiles per batch
QG = 1024            # q-group width
NQG = S // QG        # 2 q-groups per batch

BF16 = ml_dtypes.bfloat16

_prog = None


def _build_program():
    import concourse.bacc as bacc
    import concourse.tile as tile
    from concourse import mybir

    f32 = mybir.dt.float32
    bf16 = mybir.dt.bfloat16
    AF = mybir.ActivationFunctionType

    nc = bacc.Bacc("TRN2", debug=False, enable_asserts=False, num_devices=NCORES)

    xT = nc.dram_tensor("xT", [128, KC, BS], bf16, kind="ExternalInput").ap()
    wq = nc.dram_tensor("wq", [128, KC, CPC], bf16, kind="ExternalInput").ap()
    wk = nc.dram_tensor("wk", [128, KC, CPC], bf16, kind="ExternalInput").ap()
    wv = nc.dram_tensor("wv", [128, KC, CPC], bf16, kind="ExternalInput").ap()
    wo = nc.dram_tensor("wo", [CPC, D], bf16, kind="ExternalInput").ap()
    bq = nc.dram_tensor("bq", [CPC, 1], f32, kind="ExternalInput").ap()
    bk = nc.dram_tensor("bk", [CPC, 1], f32, kind="ExternalInput").ap()
    bv = nc.dram_tensor("bv", [1, CPC], bf16, kind="ExternalInput").ap()
    out = nc.dram_tensor("out", [BS, D], bf16, kind="ExternalOutput").ap()

    SCALE = float(1.0 / np.sqrt(HD))

    with tile.TileContext(nc) as tc:
        with (
            tc.tile_pool(name="big", bufs=1) as big,
            tc.tile_pool(name="sm", bufs=1) as sm,
            tc.tile_pool(name="attn", bufs=2) as attn,
            tc.tile_pool(name="etp", bufs=2) as etp,
            tc.tile_pool(name="ostage", bufs=2) as ostage,
            tc.tile_pool(name="ps", bufs=2, space="PSUM") as ps,
        ):
            # ---- resident SBUF tensors ----
            xt_sb = big.tile([128, KC, BS], bf16, name="xt_sb", tag="xt")
            qt_sb = big.tile([128, BS], bf16, name="qt_sb", tag="qt")
            kt_sb = big.tile([128, BS], bf16, name="kt_sb", tag="kt")
            # V|ones per head: [keys(128), keytile(32), head(2), 64 V + 1 ones]
            v_sb = big.tile([128, B * NKT, HPC, HD + 1], bf16, name="v_sb", tag="v")
            wo_sb = big.tile([128, D], bf16, name="wo_sb", tag="wo")

            wq_sb = sm.tile([128, KC, CPC], bf16, name="wq_sb", tag="wq")
            wk_sb = sm.tile([128, KC, CPC], bf16, name="wk_sb", tag="wk")
            wv_sb = sm.tile([128, KC, CPC], bf16, name="wv_sb", tag="wv")
            bq_sb = sm.tile([CPC, 1], f32, name="bq_sb", tag="bq")
            bk_sb = sm.tile([CPC, 1], f32, name="bk_sb", tag="bk")
            bv_sb = sm.tile([1, CPC], bf16, name="bv_sb", tag="bv")
            ones_bf = sm.tile([1, 128], bf16, name="ones_bf", tag="onesb")

            # flat [128, kt*head, HD+1] view for memset / projection evacuation
            v3 = v_sb.rearrange("p k h c -> p (k h) c")
            nc.vector.memset(ones_bf, 1.0)
            nc.vector.memset(v3[:, :, HD : HD + 1], 1.0)

            # ---- DMAs, ordered so the first projection can start ASAP ----
            nc.sync.dma_start(out=wq_sb, in_=wq)
            for c in range(KC):
                nc.sync.dma_start(out=xt_sb[:, c, 0:QG], in_=xT[:, c, 0:QG])
            nc.sync.dma_start(out=wk_sb, in_=wk)
            nc.sync.dma_start(out=xt_sb[:, :, QG : 2 * QG], in_=xT[:, :, QG : 2 * QG])
            nc.sync.dma_start(out=wv_sb, in_=wv)
            nc.sync.dma_start(out=bq_sb, in_=bq)
            nc.sync.dma_start(out=bk_sb, in_=bk)
            nc.sync.dma_start(out=bv_sb, in_=bv)
            nc.sync.dma_start(out=wo_sb, in_=wo)
            nc.sync.dma_start(out=xt_sb[:, :, 2 * QG : 3 * QG], in_=xT[:, :, 2 * QG : 3 * QG])
            nc.sync.dma_start(out=xt_sb[:, :, 3 * QG : 4 * QG], in_=xT[:, :, 3 * QG : 4 * QG])

            # ---- filler machinery: PE work interleaved into attention ----
            filler_q = collections.deque()

            def drain(n):
                done = 0
                while filler_q and done < n:
                    try:
                        next(filler_q[0])
                        done += 1
                    except StopIteration:
                        filler_q.popleft()

            def gen_qkproj(dst, w_sb, b_sb, tb, nm):
                """One 1024-token block of a Q^T/K^T projection (2 chunks)."""
                pp = ps.tile([128, QG], f32, name=f"pp_{nm}", tag="sp")
                for qh in range(2):
                    for c in range(KC):
                        nc.tensor.matmul(
                            pp[:, qh * QCH : (qh + 1) * QCH],
                            lhsT=w_sb[:, c, :],
                            rhs=xt_sb[:, c, tb * QG + qh * QCH : tb * QG + (qh + 1) * QCH],
                            start=(c == 0),
                            stop=(c == KC - 1),
                        )
                    yield
                nc.vector.tensor_scalar_add(dst[:, tb * QG : (tb + 1) * QG], pp, b_sb)

            def gen_vproj(b):
                """V projection (+bias) for batch b, natural [keys, cols]."""
                for half in range(2):
                    vp = ps.tile([128, QG], f32, name=f"vp{b}{half}", tag="sp")
                    for k8 in range(8):
                        kt = b * NKT + half * 8 + k8
                        sl = slice(k8 * 128, (k8 + 1) * 128)
                        for c in range(KC):
                            nc.tensor.matmul(
                                vp[:, sl],
                                lhsT=xt_sb[:, c, kt * 128 : (kt + 1) * 128],
                                rhs=wv_sb[:, c, :],
                                start=(c == 0),
                                stop=False,
                            )
                        nc.tensor.matmul(vp[:, sl], lhsT=ones_bf, rhs=bv_sb, start=False, stop=True)
                        nc.vector.tensor_copy(
                            v3[:, kt * HPC : (kt + 1) * HPC, 0:HD],
                            vp[:, sl].rearrange("p (h c) -> p h c", c=HD),
                        )
                        yield

            def gen_outproj(b, qt0, nqt, ot, tail=False):
                """Partial out-projection for a run of q-tiles."""
                for j, qt in enumerate(range(qt0, qt0 + nqt)):
                    pq = ps.tile([128, QG], f32, name=f"pq{b}{qt}", tag="sp")
                    for nh in range(2):
                        nc.tensor.matmul(
                            pq[:, nh * QCH : (nh + 1) * QCH],
                            lhsT=ot[:, qt * 128 : (qt + 1) * 128],
                            rhs=wo_sb[:, nh * QCH : (nh + 1) * QCH],
                            start=True,
                            stop=True,
                        )
                    os_ = ostage.tile([128, QG], bf16, name=f"os{b}{qt}", tag="os", bufs=3)
                    if tail and j % 2 == 0:
                        nc.scalar.copy(os_, pq)
                    else:
                        nc.vector.tensor_copy(os_, pq)
                    nc.sync.dma_start(
                        out=out[b * S + qt * 128 : b * S + (qt + 1) * 128, :], in_=os_
                    )
                    yield

            # ---- prefix: minimum projections before attention(b0, qg0) ----
            for g in (
                gen_qkproj(qt_sb, wq_sb, bq_sb, 0, "q0"),
                gen_qkproj(kt_sb, wk_sb, bk_sb, 0, "k0"),
                gen_qkproj(kt_sb, wk_sb, bk_sb, 1, "k1"),
            ):
                for _ in g:
                    pass
            vgen0 = gen_vproj(0)
            for _ in range(4):
                next(vgen0)
            filler_q.append(vgen0)
            filler_q.append(gen_qkproj(qt_sb, wq_sb, bq_sb, 1, "q1"))
            filler_q.append(gen_qkproj(kt_sb, wk_sb, bk_sb, 2, "k2"))
            filler_q.append(gen_qkproj(kt_sb, wk_sb, bk_sb, 3, "k3"))
            filler_q.append(gen_qkproj(qt_sb, wq_sb, bq_sb, 2, "q2"))
            filler_q.append(gen_vproj(1))
            filler_q.append(gen_qkproj(qt_sb, wq_sb, bq_sb, 3, "q3"))

            # ---- attention, ACT-paced; PE idle slots consumed by fillers ----
            ot_tiles = {}
            for b in range(B):
                ot_tiles[b] = attn.tile([128, S], bf16, name=f"ot{b}", tag="ot")
            for b in range(B):
                ot = ot_tiles[b]
                for qg in range(NQG):
                    q0 = b * S + qg * QG
                    et = [
                        etp.tile([128, NKT, QG], bf16, name=f"et{b}{qg}{h}", tag="et")
                        for h in range(HPC)
                    ]
                    op = [
                        ps.tile([HD + 1, QG], f32, name=f"op{b}{qg}{h}", tag="op")
                        for h in range(HPC)
                    ]
                    # attnV for kt is emitted DELAY slots later so the PE
                    # stream doesn't block on the previous q-group's PSUM
                    # accumulators while its normalize chain drains on DVE.
                    DELAY = 3
                    attnv_pending = collections.deque()

                    def emit_attnv(kt):
                        for h in range(HPC):
                            for qc in range(2):
                                nc.tensor.matmul(
                                    op[h][:, qc * QCH : (qc + 1) * QCH],
                                    lhsT=v_sb[:, b * NKT + kt, h, :],
                                    rhs=et[h][:, kt, qc * QCH : (qc + 1) * QCH],
                                    start=(kt == 0),
                                    stop=(kt == NKT - 1),
                                )

                    for kt in range(NKT):
                        sps = [
                            ps.tile([128, QG], f32, name=f"sp{b}{qg}{h}{kt}", tag="sp")
                            for h in range(HPC)
                        ]
                        # head0 rows 0-63, head1 rows 64-127: adjacent issue ->
                        # the pair co-executes on disjoint PE row groups
                        for qh in range(2):
                            for h in range(HPC):
                                hp = h * HD
                                nc.tensor.matmul(
                                    sps[h][:, qh * QCH : (qh + 1) * QCH],
                                    lhsT=kt_sb[hp : hp + HD, b * S + kt * 128 : b * S + (kt + 1) * 128],
                                    rhs=qt_sb[hp : hp + HD, q0 + qh * QCH : q0 + (qh + 1) * QCH],
                                    start=True,
                                    stop=True,
                                )
                        for h in range(HPC):
                            nc.scalar.activation(et[h][:, kt, :], sps[h], AF.Exp, scale=SCALE)
                        attnv_pending.append(kt)
                        if kt >= DELAY:
                            emit_attnv(attnv_pending.popleft())
                        drain(2)
                    while attnv_pending:
                        emit_attnv(attnv_pending.popleft())
                    # normalize per 512-col half: recip (DVE) -> broadcast
                    # (GpSimd) -> mul (DVE); out-projections chase each half
                    last = b == B - 1 and qg == NQG - 1
                    for qc in range(2):
                        qsl = slice(qc * QCH, (qc + 1) * QCH)
                        rcs, rbss = [], []
                        for h in range(HPC):
                            rc = ostage.tile([1, QCH], f32, name=f"rc{b}{qg}{h}{qc}", tag="rc", bufs=2)
                            nc.vector.reciprocal(rc, op[h][HD : HD + 1, qsl])
                            rcs.append(rc)
                        for h in range(HPC):
                            rbs = ostage.tile([HD, QCH], f32, name=f"rbs{b}{qg}{h}{qc}", tag="rbs", bufs=2)
                            nc.gpsimd.partition_broadcast(rbs, rcs[h])
                            rbss.append(rbs)
                        for h in range(HPC):
                            hp = h * HD
                            nc.vector.tensor_mul(
                                ot[hp : hp + HD, qg * QG + qc * QCH : qg * QG + (qc + 1) * QCH],
                                op[h][0:HD, qsl],
                                rbss[h],
                            )
                        filler_q.append(
                            gen_outproj(b, qg * 8 + qc * 4, 4, ot, tail=last)
                        )
            drain(10000)

    nc.compile()
    return nc


def _get_prog():
    global _prog
    if _prog is None:
        _prog = _build_program()
    return _prog


def _perm_ckc(a):
    """[D, N] -> [128, KC, N] with partition-contiguous rows."""
    return np.ascontiguousarray(a.reshape(KC, 128, -1).transpose(1, 0, 2))


def kernel(x, Wq, bq, Wk, bk, Wv, bv, Wo, bo):
    from concourse import bass_utils

    nc = _get_prog()

    xT = np.asarray(x, dtype=np.float32).reshape(BS, D).T.astype(BF16)
    xTp = _perm_ckc(xT)

    in_maps = []
    for c in range(NCORES):
        cols = slice(c * CPC, (c + 1) * CPC)
        in_maps.append(
            {
                "xT": xTp,
                "wq": _perm_ckc(Wq[cols, :].T.astype(BF16)),
                "wk": _perm_ckc(Wk[cols, :].T.astype(BF16)),
                "wv": _perm_ckc(Wv[cols, :].T.astype(BF16)),
                "wo": np.ascontiguousarray(Wo[:, cols].T).astype(BF16),
                "bq": np.asarray(bq[cols], np.float32).reshape(CPC, 1),
                "bk": np.asarray(bk[cols], np.float32).reshape(CPC, 1),
                "bv": np.asarray(bv[cols], np.float32).reshape(1, CPC).astype(BF16),
            }
        )

    res = bass_utils.run_bass_kernel_spmd(
        nc,
        in_maps,
        core_ids=list(range(NCORES)),
        trace=bool(int(os.environ.get("KERNEL_TRACE", "0"))),
    )
    kernel.last_results = res

    acc = np.zeros((BS, D), np.float64)
    for c in range(NCORES):
        acc += res.results[c]["out"].astype(np.float64)
    acc += np.asarray(bo, np.float64)[None, :]
    return acc.reshape(B, S, D).astype(np.float32)


# revision 10
# speedup vs baseline: 1.0419x; 1.0248x over previous
"""Multi-head attention (B=2, S=2048, D=1024, H=16) on 8 NeuronCores.

Sharding: tensor-parallel over heads — 2 heads per core. Each core computes
q/k/v projections for its 128 output columns, full attention for its 2 heads
(both batches), and a partial out-projection [4096, 1024] in bf16. Host sums
the 8 partials and adds the output bias.

Schedule (single fused pipeline, ACT-exp is the long pole):
  - Q^T/K^T/V projections and deferred out-projections are emitted as
    "filler" PE work interleaved into the attention kt-loop, which is paced
    by the ACT engine's exp throughput. Attention for batch 0 starts as soon
    as Q(b0,qg0) + K(b0) are projected; everything else fills PE idle slots.
  - scores for the two heads are issued back-to-back with separate PSUM
    tiles; head0 occupies PE rows 0-63, head1 rows 64-127 (tile_position
    auto-derived), so the two matmuls co-execute on disjoint row groups.
  - V carries an extra all-ones column so attn@[V|1] yields the softmax
    denominator (row 64) along with the unnormalized output (rows 0..63).
  - softmax skips max-subtraction: scores are ~N(0, 0.33^2) by construction.
  - denominators: DVE reciprocal_approx_fast -> GpSimd partition_broadcast
    -> DVE multiply during PSUM evacuation.
  - weights / xT are host-permuted so every DMA is 2KB-contiguous per
    partition.
"""

import collections
import os

import ml_dtypes
import numpy as np

B, S, D, H = 2, 2048, 1024, 16
HD = D // H          # 64
BS = B * S           # 4096 tokens
NCORES = 8
HPC = H // NCORES    # heads per core = 2
CPC = HPC * HD       # output cols per core = 128
KC = D // 128        # contract chunks = 8
QCH = 512            # matmul moving free dim
NKT = S // 128       # 16 key tiles per batch
QG = 1024            # q-group width
NQG = S // QG        # 2 q-groups per batch

BF16 = ml_dtypes.bfloat16

_prog = None


def _build_program():
    import concourse.bacc as bacc
    import concourse.tile as tile
    from concourse import mybir

    f32 = mybir.dt.float32
    bf16 = mybir.dt.bfloat16
    AF = mybir.ActivationFunctionType

    nc = bacc.Bacc("TRN2", debug=False, enable_asserts=False, num_devices=NCORES)

    xT = nc.dram_tensor("xT", [128, KC, BS], bf16, kind="ExternalInput").ap()
    wq = nc.dram_tensor("wq", [128, KC, CPC], bf16, kind="ExternalInput").ap()
    wk = nc.dram_tensor("wk", [128, KC, CPC], bf16, kind="ExternalInput").ap()
    wv = nc.dram_tensor("wv", [128, KC, CPC], bf16, kind="ExternalInput").ap()
    wo = nc.dram_tensor("wo", [CPC, D], bf16, kind="ExternalInput").ap()
    bq = nc.dram_tensor("bq", [CPC, 1], f32, kind="ExternalInput").ap()
    bk = nc.dram_tensor("bk", [CPC, 1], f32, kind="ExternalInput").ap()
    bv = nc.dram_tensor("bv", [1, CPC], bf16, kind="ExternalInput").ap()
    out = nc.dram_tensor("out", [BS, D], bf16, kind="ExternalOutput").ap()

    SCALE = float(1.0 / np.sqrt(HD))

    with tile.TileContext(nc) as tc:
        with (
            tc.tile_pool(name="big", bufs=1) as big,
            tc.tile_pool(name="sm", bufs=1) as sm,
            tc.tile_pool(name="attn", bufs=2) as attn,
            tc.tile_pool(name="etp", bufs=2) as etp,
            tc.tile_pool(name="ostage", bufs=2) as ostage,
            tc.tile_pool(name="ps", bufs=2, space="PSUM") as ps,
        ):
            # ---- resident SBUF tensors ----
            xt_sb = big.tile([128, KC, BS], bf16, name="xt_sb", tag="xt")
            qt_sb = big.tile([128, BS], bf16, name="qt_sb", tag="qt")
            kt_sb = big.tile([128, BS], bf16, name="kt_sb", tag="kt")
            # V|ones per head: [keys(128), keytile(32), head(2), 64 V + 1 ones]
            v_sb = big.tile([128, B * NKT, HPC, HD + 1], bf16, name="v_sb", tag="v")
            wo_sb = big.tile([128, D], bf16, name="wo_sb", tag="wo")

            wq_sb = sm.tile([128, KC, CPC], bf16, name="wq_sb", tag="wq")
            wk_sb = sm.tile([128, KC, CPC], bf16, name="wk_sb", tag="wk")
            wv_sb = sm.tile([128, KC, CPC], bf16, name="wv_sb", tag="wv")
            bq_sb = sm.tile([CPC, 1], f32, name="bq_sb", tag="bq")
            bk_sb = sm.tile([CPC, 1], f32, name="bk_sb", tag="bk")
            bv_sb = sm.tile([1, CPC], bf16, name="bv_sb", tag="bv")
            ones_bf = sm.tile([1, 128], bf16, name="ones_bf", tag="onesb")

            # flat [128, kt*head, HD+1] view for memset / projection evacuation
            v3 = v_sb.rearrange("p k h c -> p (k h) c")
            nc.vector.memset(ones_bf, 1.0)
            nc.vector.memset(v3[:, :, HD : HD + 1], 1.0)

            # ---- DMAs, ordered so the first projection can start ASAP ----
            nc.sync.dma_start(out=wq_sb, in_=wq)
            for c in range(KC):
                nc.sync.dma_start(out=xt_sb[:, c, 0:QG], in_=xT[:, c, 0:QG])
            nc.sync.dma_start(out=wk_sb, in_=wk)
            nc.sync.dma_start(out=xt_sb[:, :, QG : 2 * QG], in_=xT[:, :, QG : 2 * QG])
            nc.sync.dma_start(out=wv_sb, in_=wv)
            nc.sync.dma_start(out=bq_sb, in_=bq)
            nc.sync.dma_start(out=bk_sb, in_=bk)
            nc.sync.dma_start(out=bv_sb, in_=bv)
            nc.sync.dma_start(out=wo_sb, in_=wo)
            nc.sync.dma_start(out=xt_sb[:, :, 2 * QG : 3 * QG], in_=xT[:, :, 2 * QG : 3 * QG])
            nc.sync.dma_start(out=xt_sb[:, :, 3 * QG : 4 * QG], in_=xT[:, :, 3 * QG : 4 * QG])

            # ---- filler machinery: PE work interleaved into attention ----
            filler_q = collections.deque()

            def drain(n):
                done = 0
                while filler_q and done < n:
                    try:
                        next(filler_q[0])
                        done += 1
                    except StopIteration:
                        filler_q.popleft()

            def gen_qkproj(dst, w_sb, b_sb, tb, nm):
                """One 1024-token block of a Q^T/K^T projection (2 chunks)."""
                pp = ps.tile([128, QG], f32, name=f"pp_{nm}", tag="sp")
                for qh in range(2):
                    for c in range(KC):
                        nc.tensor.matmul(
                            pp[:, qh * QCH : (qh + 1) * QCH],
                            lhsT=w_sb[:, c, :],
                            rhs=xt_sb[:, c, tb * QG + qh * QCH : tb * QG + (qh + 1) * QCH],
                            start=(c == 0),
                            stop=(c == KC - 1),
                        )
                    yield
                nc.vector.tensor_scalar_add(dst[:, tb * QG : (tb + 1) * QG], pp, b_sb)

            def gen_vproj(b):
                """V projection (+bias) for batch b, natural [keys, cols]."""
                for half in range(2):
                    vp = ps.tile([128, QG], f32, name=f"vp{b}{half}", tag="sp")
                    for k8 in range(8):
                        kt = b * NKT + half * 8 + k8
                        sl = slice(k8 * 128, (k8 + 1) * 128)
                        for c in range(KC):
                            nc.tensor.matmul(
                                vp[:, sl],
                                lhsT=xt_sb[:, c, kt * 128 : (kt + 1) * 128],
                                rhs=wv_sb[:, c, :],
                                start=(c == 0),
                                stop=False,
                            )
                        nc.tensor.matmul(vp[:, sl], lhsT=ones_bf, rhs=bv_sb, start=False, stop=True)
                        nc.vector.tensor_copy(
                            v3[:, kt * HPC : (kt + 1) * HPC, 0:HD],
                            vp[:, sl].rearrange("p (h c) -> p h c", c=HD),
                        )
                        yield

            def gen_outproj(b, qt0, nqt, ot, tail=False):
                """Partial out-projection for a run of q-tiles."""
                for j, qt in enumerate(range(qt0, qt0 + nqt)):
                    pq = ps.tile([128, QG], f32, name=f"pq{b}{qt}", tag="sp")
                    for nh in range(2):
                        nc.tensor.matmul(
                            pq[:, nh * QCH : (nh + 1) * QCH],
                            lhsT=ot[:, qt * 128 : (qt + 1) * 128],
                            rhs=wo_sb[:, nh * QCH : (nh + 1) * QCH],
                            start=True,
                            stop=True,
                        )
                    os_ = ostage.tile([128, QG], bf16, name=f"os{b}{qt}", tag="os", bufs=3)
                    if tail and j % 2 == 0:
                        nc.scalar.copy(os_, pq)
                    else:
                        nc.vector.tensor_copy(os_, pq)
                    nc.sync.dma_start(
                        out=out[b * S + qt * 128 : b * S + (qt + 1) * 128, :], in_=os_
                    )
                    yield

            # ---- prefix: minimum projections before attention(b0, qg0) ----
            for g in (
                gen_qkproj(qt_sb, wq_sb, bq_sb, 0, "q0"),
                gen_qkproj(kt_sb, wk_sb, bk_sb, 0, "k0"),
                gen_qkproj(kt_sb, wk_sb, bk_sb, 1, "k1"),
            ):
                for _ in g:
                    pass
            vgen0 = gen_vproj(0)
            for _ in range(4):
                next(vgen0)
            filler_q.append(vgen0)
            filler_q.append(gen_qkproj(qt_sb, wq_sb, bq_sb, 1, "q1"))
            filler_q.append(gen_qkproj(kt_sb, wk_sb, bk_sb, 2, "k2"))
            filler_q.append(gen_qkproj(kt_sb, wk_sb, bk_sb, 3, "k3"))
            filler_q.append(gen_qkproj(qt_sb, wq_sb, bq_sb, 2, "q2"))
            filler_q.append(gen_vproj(1))
            filler_q.append(gen_qkproj(qt_sb, wq_sb, bq_sb, 3, "q3"))

            # ---- attention, ACT-paced; PE idle slots consumed by fillers ----
            ot_tiles = {}
            for b in range(B):
                ot_tiles[b] = attn.tile([128, S], bf16, name=f"ot{b}", tag="ot")
            for b in range(B):
                ot = ot_tiles[b]
                for qg in range(NQG):
                    q0 = b * S + qg * QG
                    et = [
                        etp.tile([128, NKT, QG], bf16, name=f"et{b}{qg}{h}", tag="et")
                        for h in range(HPC)
                    ]
                    # both heads' attn@[V|1] accumulators in ONE psum tile so a
                    # single flat-cost DVE reciprocal covers every denominator
                    op_all = ps.tile([HD + 1, HPC * QG], f32, name=f"op{b}{qg}", tag="op", bufs=1)
                    op = [op_all[:, h * QG : (h + 1) * QG] for h in range(HPC)]
                    # attnV for kt is emitted DELAY slots later so the PE
                    # stream doesn't block on the previous q-group's PSUM
                    # accumulators while its normalize chain drains on DVE.
                    DELAY = 3
                    attnv_pending = collections.deque()

                    def emit_attnv(kt):
                        for h in range(HPC):
                            for qc in range(2):
                                nc.tensor.matmul(
                                    op[h][:, qc * QCH : (qc + 1) * QCH],
                                    lhsT=v_sb[:, b * NKT + kt, h, :],
                                    rhs=et[h][:, kt, qc * QCH : (qc + 1) * QCH],
                                    start=(kt == 0),
                                    stop=(kt == NKT - 1),
                                )

                    for kt in range(NKT):
                        sps = [
                            ps.tile([128, QG], f32, name=f"sp{b}{qg}{h}{kt}", tag="sp")
                            for h in range(HPC)
                        ]
                        # head0 rows 0-63, head1 rows 64-127: adjacent issue ->
                        # the pair co-executes on disjoint PE row groups
                        for qh in range(2):
                            for h in range(HPC):
                                hp = h * HD
                                nc.tensor.matmul(
                                    sps[h][:, qh * QCH : (qh + 1) * QCH],
                                    lhsT=kt_sb[hp : hp + HD, b * S + kt * 128 : b * S + (kt + 1) * 128],
                                    rhs=qt_sb[hp : hp + HD, q0 + qh * QCH : q0 + (qh + 1) * QCH],
                                    start=True,
                                    stop=True,
                                )
                        for h in range(HPC):
                            nc.scalar.activation(et[h][:, kt, :], sps[h], AF.Exp, scale=SCALE)
                        attnv_pending.append(kt)
                        if kt >= DELAY:
                            emit_attnv(attnv_pending.popleft())
                        drain(2)
                    while attnv_pending:
                        emit_attnv(attnv_pending.popleft())
                    # normalize: ONE recip (DVE, flat cost) -> per-head
                    # broadcast (GpSimd) -> per-head mul (DVE)
                    last = b == B - 1 and qg == NQG - 1
                    rc = ostage.tile([1, HPC * QG], f32, name=f"rc{b}{qg}", tag="rc", bufs=2)
                    nc.vector.reciprocal(rc, op_all[HD : HD + 1, :])
                    rbss = []
                    for h in range(HPC):
                        rbs = ostage.tile([HD, QG], f32, name=f"rbs{b}{qg}{h}", tag="rbs", bufs=2)
                        nc.gpsimd.partition_broadcast(rbs, rc[0:1, h * QG : (h + 1) * QG])
                        rbss.append(rbs)
                    for h in range(HPC):
                        hp = h * HD
                        nc.vector.tensor_mul(
                            ot[hp : hp + HD, qg * QG : (qg + 1) * QG],
                            op[h][0:HD, :],
                            rbss[h],
                        )
                    filler_q.append(gen_outproj(b, qg * 8, 8, ot, tail=last))
            drain(10000)

    nc.compile()
    return nc


def _get_prog():
    global _prog
    if _prog is None:
        _prog = _build_program()
    return _prog


def _perm_ckc(a):
    """[D, N] -> [128, KC, N] with partition-contiguous rows."""
    return np.ascontiguousarray(a.reshape(KC, 128, -1).transpose(1, 0, 2))


def kernel(x, Wq, bq, Wk, bk, Wv, bv, Wo, bo):
    from concourse import bass_utils

    nc = _get_prog()

    xT = np.asarray(x, dtype=np.float32).reshape(BS, D).T.astype(BF16)
    xTp = _perm_ckc(xT)

    in_maps = []
    for c in range(NCORES):
        cols = slice(c * CPC, (c + 1) * CPC)
        in_maps.append(
            {
                "xT": xTp,
                "wq": _perm_ckc(Wq[cols, :].T.astype(BF16)),
                "wk": _perm_ckc(Wk[cols, :].T.astype(BF16)),
                "wv": _perm_ckc(Wv[cols, :].T.astype(BF16)),
                "wo": np.ascontiguousarray(Wo[:, cols].T).astype(BF16),
                "bq": np.asarray(bq[cols], np.float32).reshape(CPC, 1),
                "bk": np.asarray(bk[cols], np.float32).reshape(CPC, 1),
                "bv": np.asarray(bv[cols], np.float32).reshape(1, CPC).astype(BF16),
            }
        )

    res = bass_utils.run_bass_kernel_spmd(
        nc,
        in_maps,
        core_ids=list(range(NCORES)),
        trace=bool(int(os.environ.get("KERNEL_TRACE", "0"))),
    )
    kernel.last_results = res

    acc = np.zeros((BS, D), np.float64)
    for c in range(NCORES):
        acc += res.results[c]["out"].astype(np.float64)
    acc += np.asarray(bo, np.float64)[None, :]
    return acc.reshape(B, S, D).astype(np.float32)


# revision 11
# speedup vs baseline: 1.0806x; 1.0372x over previous
"""Multi-head attention (B=2, S=2048, D=1024, H=16) on 8 NeuronCores.

Sharding: tensor-parallel over heads — 2 heads per core. Each core computes
q/k/v projections for its 128 output columns, full attention for its 2 heads
(both batches), and a partial out-projection [4096, 1024] in bf16. Host sums
the 8 partials and adds the output bias.

Schedule (single fused pipeline, ACT-exp is the long pole):
  - Q^T/K^T/V projections and deferred out-projections are emitted as
    "filler" PE work interleaved into the attention kt-loop, which is paced
    by the ACT engine's exp throughput. Attention for batch 0 starts as soon
    as Q(b0,qg0) + K(b0) are projected; everything else fills PE idle slots.
  - scores for the two heads are issued back-to-back with separate PSUM
    tiles; head0 occupies PE rows 0-63, head1 rows 64-127 (tile_position
    auto-derived), so the two matmuls co-execute on disjoint row groups.
  - V carries an extra all-ones column so attn@[V|1] yields the softmax
    denominator (row 64) along with the unnormalized output (rows 0..63).
  - softmax skips max-subtraction: scores are ~N(0, 0.33^2) by construction.
  - denominators: DVE reciprocal_approx_fast -> GpSimd partition_broadcast
    -> DVE multiply during PSUM evacuation.
  - weights / xT are host-permuted so every DMA is 2KB-contiguous per
    partition.
"""

import collections
import os

import ml_dtypes
import numpy as np

B, S, D, H = 2, 2048, 1024, 16
HD = D // H          # 64
BS = B * S           # 4096 tokens
NCORES = 8
HPC = H // NCORES    # heads per core = 2
CPC = HPC * HD       # output cols per core = 128
KC = D // 128        # contract chunks = 8
QCH = 512            # matmul moving free dim
NKT = S // 128       # 16 key tiles per batch
QG = 1024            # q-group width
NQG = S // QG        # 2 q-groups per batch

BF16 = ml_dtypes.bfloat16

_prog = None


def _build_program():
    import concourse.bacc as bacc
    import concourse.tile as tile
    from concourse import mybir

    f32 = mybir.dt.float32
    bf16 = mybir.dt.bfloat16
    AF = mybir.ActivationFunctionType

    nc = bacc.Bacc("TRN2", debug=False, enable_asserts=False, num_devices=NCORES)

    xT = nc.dram_tensor("xT", [128, KC, BS], bf16, kind="ExternalInput").ap()
    wq = nc.dram_tensor("wq", [128, KC, CPC], bf16, kind="ExternalInput").ap()
    wk = nc.dram_tensor("wk", [128, KC, CPC], bf16, kind="ExternalInput").ap()
    wv = nc.dram_tensor("wv", [128, KC, CPC], bf16, kind="ExternalInput").ap()
    wo = nc.dram_tensor("wo", [CPC, D], bf16, kind="ExternalInput").ap()
    bq = nc.dram_tensor("bq", [CPC, 1], f32, kind="ExternalInput").ap()
    bk = nc.dram_tensor("bk", [CPC, 1], f32, kind="ExternalInput").ap()
    bv = nc.dram_tensor("bv", [1, CPC], bf16, kind="ExternalInput").ap()
    out = nc.dram_tensor("out", [BS, D], bf16, kind="ExternalOutput").ap()

    SCALE = float(1.0 / np.sqrt(HD))

    with tile.TileContext(nc) as tc:
        with (
            tc.tile_pool(name="big", bufs=1) as big,
            tc.tile_pool(name="sm", bufs=1) as sm,
            tc.tile_pool(name="attn", bufs=2) as attn,
            tc.tile_pool(name="etp", bufs=2) as etp,
            tc.tile_pool(name="ostage", bufs=2) as ostage,
            tc.tile_pool(name="ps", bufs=2, space="PSUM") as ps,
        ):
            # ---- resident SBUF tensors ----
            xt_sb = big.tile([128, KC, BS], bf16, name="xt_sb", tag="xt")
            qt_sb = big.tile([128, BS], bf16, name="qt_sb", tag="qt")
            kt_sb = big.tile([128, BS], bf16, name="kt_sb", tag="kt")
            # V|ones per head: [keys(128), keytile(32), head(2), 64 V + 1 ones]
            v_sb = big.tile([128, B * NKT, HPC, HD + 1], bf16, name="v_sb", tag="v")
            wo_sb = big.tile([128, D], bf16, name="wo_sb", tag="wo")

            wq_sb = sm.tile([128, KC, CPC], bf16, name="wq_sb", tag="wq")
            wk_sb = sm.tile([128, KC, CPC], bf16, name="wk_sb", tag="wk")
            wv_sb = sm.tile([128, KC, CPC], bf16, name="wv_sb", tag="wv")
            bq_sb = sm.tile([CPC, 1], f32, name="bq_sb", tag="bq")
            bk_sb = sm.tile([CPC, 1], f32, name="bk_sb", tag="bk")
            bv_sb = sm.tile([1, CPC], bf16, name="bv_sb", tag="bv")
            ones_bf = sm.tile([1, 128], bf16, name="ones_bf", tag="onesb")

            # flat [128, kt*head, HD+1] view for memset / projection evacuation
            v3 = v_sb.rearrange("p k h c -> p (k h) c")
            nc.vector.memset(ones_bf, 1.0)
            nc.vector.memset(v3[:, :, HD : HD + 1], 1.0)

            # ---- DMAs, ordered so the first projection can start ASAP ----
            nc.sync.dma_start(out=wq_sb, in_=wq)
            for c in range(KC):
                nc.sync.dma_start(out=xt_sb[:, c, 0:QG], in_=xT[:, c, 0:QG])
            nc.sync.dma_start(out=wk_sb, in_=wk)
            nc.sync.dma_start(out=xt_sb[:, :, QG : 2 * QG], in_=xT[:, :, QG : 2 * QG])
            nc.sync.dma_start(out=wv_sb, in_=wv)
            nc.sync.dma_start(out=bq_sb, in_=bq)
            nc.sync.dma_start(out=bk_sb, in_=bk)
            nc.sync.dma_start(out=bv_sb, in_=bv)
            nc.sync.dma_start(out=wo_sb, in_=wo)
            nc.sync.dma_start(out=xt_sb[:, :, 2 * QG : 3 * QG], in_=xT[:, :, 2 * QG : 3 * QG])
            nc.sync.dma_start(out=xt_sb[:, :, 3 * QG : 4 * QG], in_=xT[:, :, 3 * QG : 4 * QG])

            # ---- filler machinery: PE work interleaved into attention ----
            filler_q = collections.deque()

            def drain(n):
                done = 0
                while filler_q and done < n:
                    try:
                        next(filler_q[0])
                        done += 1
                    except StopIteration:
                        filler_q.popleft()

            def gen_qkproj(dst, w_sb, b_sb, tb, nm):
                """One 1024-token block of a Q^T/K^T projection (2 chunks)."""
                pp = ps.tile([128, QG], f32, name=f"pp_{nm}", tag="sp")
                for qh in range(2):
                    for c in range(KC):
                        nc.tensor.matmul(
                            pp[:, qh * QCH : (qh + 1) * QCH],
                            lhsT=w_sb[:, c, :],
                            rhs=xt_sb[:, c, tb * QG + qh * QCH : tb * QG + (qh + 1) * QCH],
                            start=(c == 0),
                            stop=(c == KC - 1),
                        )
                    yield
                nc.vector.tensor_scalar_add(dst[:, tb * QG : (tb + 1) * QG], pp, b_sb)

            def gen_vproj(b):
                """V projection (+bias) for batch b, natural [keys, cols]."""
                for half in range(2):
                    vp = ps.tile([128, QG], f32, name=f"vp{b}{half}", tag="sp")
                    for k8 in range(8):
                        kt = b * NKT + half * 8 + k8
                        sl = slice(k8 * 128, (k8 + 1) * 128)
                        for c in range(KC):
                            nc.tensor.matmul(
                                vp[:, sl],
                                lhsT=xt_sb[:, c, kt * 128 : (kt + 1) * 128],
                                rhs=wv_sb[:, c, :],
                                start=(c == 0),
                                stop=False,
                            )
                        nc.tensor.matmul(vp[:, sl], lhsT=ones_bf, rhs=bv_sb, start=False, stop=True)
                        nc.vector.tensor_copy(
                            v3[:, kt * HPC : (kt + 1) * HPC, 0:HD],
                            vp[:, sl].rearrange("p (h c) -> p h c", c=HD),
                        )
                        yield

            def gen_outproj(b, qt0, nqt, ot, tail=False):
                """Partial out-projection for a run of q-tiles."""
                for j, qt in enumerate(range(qt0, qt0 + nqt)):
                    pq = ps.tile([128, QG], f32, name=f"pq{b}{qt}", tag="sp")
                    for nh in range(2):
                        nc.tensor.matmul(
                            pq[:, nh * QCH : (nh + 1) * QCH],
                            lhsT=ot[:, qt * 128 : (qt + 1) * 128],
                            rhs=wo_sb[:, nh * QCH : (nh + 1) * QCH],
                            start=True,
                            stop=True,
                        )
                    os_ = ostage.tile([128, QG], bf16, name=f"os{b}{qt}", tag="os", bufs=3)
                    if tail and j % 2 == 0:
                        nc.scalar.copy(os_, pq)
                    else:
                        nc.vector.tensor_copy(os_, pq)
                    nc.sync.dma_start(
                        out=out[b * S + qt * 128 : b * S + (qt + 1) * 128, :], in_=os_
                    )
                    yield

            # ---- prefix: minimum projections before attention(b0, qg0) ----
            for g in (
                gen_qkproj(qt_sb, wq_sb, bq_sb, 0, "q0"),
                gen_qkproj(kt_sb, wk_sb, bk_sb, 0, "k0"),
                gen_qkproj(kt_sb, wk_sb, bk_sb, 1, "k1"),
            ):
                for _ in g:
                    pass
            vgen0 = gen_vproj(0)
            for _ in range(4):
                next(vgen0)
            filler_q.append(vgen0)
            filler_q.append(gen_qkproj(qt_sb, wq_sb, bq_sb, 1, "q1"))
            filler_q.append(gen_qkproj(kt_sb, wk_sb, bk_sb, 2, "k2"))
            filler_q.append(gen_qkproj(kt_sb, wk_sb, bk_sb, 3, "k3"))
            filler_q.append(gen_qkproj(qt_sb, wq_sb, bq_sb, 2, "q2"))
            filler_q.append(gen_vproj(1))
            filler_q.append(gen_qkproj(qt_sb, wq_sb, bq_sb, 3, "q3"))

            # ---- attention, ACT-paced; PE idle slots consumed by fillers ----
            # The normalize chain for q-group G (recip -> broadcast -> mul) is
            # emitted during q-group G+1's first kt slots, after its PE deps
            # already ran, so it never head-of-line-blocks the DVE FIFO.
            DELAY = 4
            ot_tiles = {}
            for b in range(B):
                ot_tiles[b] = attn.tile([128, S], bf16, name=f"ot{b}", tag="ot")

            def make_norm_units(b, qg, ot, op_all, op):
                last = b == B - 1 and qg == NQG - 1
                state = {}

                def u_recip():
                    rc = ostage.tile([1, HPC * QG], f32, name=f"rc{b}{qg}", tag="rc", bufs=2)
                    nc.vector.reciprocal(rc, op_all[HD : HD + 1, :])
                    state["rc"] = rc

                def u_bcast():
                    state["rbs"] = []
                    for h in range(HPC):
                        rbs = ostage.tile([HD, QG], f32, name=f"rbs{b}{qg}{h}", tag="rbs", bufs=2)
                        nc.gpsimd.partition_broadcast(
                            rbs, state["rc"][0:1, h * QG : (h + 1) * QG]
                        )
                        state["rbs"].append(rbs)

                def u_mul(h):
                    hp = h * HD
                    nc.vector.tensor_mul(
                        ot[hp : hp + HD, qg * QG : (qg + 1) * QG],
                        op[h][0:HD, :],
                        state["rbs"][h],
                    )
                    if h == HPC - 1:
                        filler_q.append(
                            gen_outproj(b, qg * 8, 8, ot, tail=last)
                        )

                return collections.deque(
                    [u_recip, u_bcast, lambda: u_mul(0), lambda: u_mul(1)]
                )

            pending_norm = collections.deque()
            for b in range(B):
                ot = ot_tiles[b]
                for qg in range(NQG):
                    q0 = b * S + qg * QG
                    et = [
                        etp.tile([128, NKT, QG], bf16, name=f"et{b}{qg}{h}", tag="et")
                        for h in range(HPC)
                    ]
                    # both heads' attn@[V|1] accumulators in ONE psum tile so a
                    # single flat-cost DVE reciprocal covers every denominator;
                    # allocated lazily (WAR on the previous group's normalize)
                    opref = {}

                    def emit_attnv(kt):
                        if "op" not in opref:
                            op_all = ps.tile(
                                [HD + 1, HPC * QG], f32, name=f"op{b}{qg}", tag="op", bufs=1
                            )
                            opref["op"] = op_all
                            opref["heads"] = [
                                op_all[:, h * QG : (h + 1) * QG] for h in range(HPC)
                            ]
                        for h in range(HPC):
                            for qc in range(2):
                                nc.tensor.matmul(
                                    opref["heads"][h][:, qc * QCH : (qc + 1) * QCH],
                                    lhsT=v_sb[:, b * NKT + kt, h, :],
                                    rhs=et[h][:, kt, qc * QCH : (qc + 1) * QCH],
                                    start=(kt == 0),
                                    stop=(kt == NKT - 1),
                                )

                    attnv_pending = collections.deque()
                    for kt in range(NKT):
                        sps = [
                            ps.tile([128, QG], f32, name=f"sp{b}{qg}{h}{kt}", tag="sp")
                            for h in range(HPC)
                        ]
                        # head0 rows 0-63, head1 rows 64-127: adjacent issue ->
                        # the pair co-executes on disjoint PE row groups
                        for qh in range(2):
                            for h in range(HPC):
                                hp = h * HD
                                nc.tensor.matmul(
                                    sps[h][:, qh * QCH : (qh + 1) * QCH],
                                    lhsT=kt_sb[hp : hp + HD, b * S + kt * 128 : b * S + (kt + 1) * 128],
                                    rhs=qt_sb[hp : hp + HD, q0 + qh * QCH : q0 + (qh + 1) * QCH],
                                    start=True,
                                    stop=True,
                                )
                        for h in range(HPC):
                            nc.scalar.activation(et[h][:, kt, :], sps[h], AF.Exp, scale=SCALE)
                        attnv_pending.append(kt)
                        if kt >= DELAY:
                            emit_attnv(attnv_pending.popleft())
                        if pending_norm:
                            pending_norm.popleft()()
                            drain(1)
                        else:
                            drain(2)
                    while attnv_pending:
                        emit_attnv(attnv_pending.popleft())
                    pending_norm = make_norm_units(b, qg, ot, opref["op"], opref["heads"])
            while pending_norm:
                pending_norm.popleft()()
            drain(10000)

    nc.compile()
    return nc


def _get_prog():
    global _prog
    if _prog is None:
        _prog = _build_program()
    return _prog


def _perm_ckc(a):
    """[D, N] -> [128, KC, N] with partition-contiguous rows."""
    return np.ascontiguousarray(a.reshape(KC, 128, -1).transpose(1, 0, 2))


def kernel(x, Wq, bq, Wk, bk, Wv, bv, Wo, bo):
    from concourse import bass_utils

    nc = _get_prog()

    xT = np.asarray(x, dtype=np.float32).reshape(BS, D).T.astype(BF16)
    xTp = _perm_ckc(xT)

    in_maps = []
    for c in range(NCORES):
        cols = slice(c * CPC, (c + 1) * CPC)
        in_maps.append(
            {
                "xT": xTp,
                "wq": _perm_ckc(Wq[cols, :].T.astype(BF16)),
                "wk": _perm_ckc(Wk[cols, :].T.astype(BF16)),
                "wv": _perm_ckc(Wv[cols, :].T.astype(BF16)),
                "wo": np.ascontiguousarray(Wo[:, cols].T).astype(BF16),
                "bq": np.asarray(bq[cols], np.float32).reshape(CPC, 1),
                "bk": np.asarray(bk[cols], np.float32).reshape(CPC, 1),
                "bv": np.asarray(bv[cols], np.float32).reshape(1, CPC).astype(BF16),
            }
        )

    res = bass_utils.run_bass_kernel_spmd(
        nc,
        in_maps,
        core_ids=list(range(NCORES)),
        trace=bool(int(os.environ.get("KERNEL_TRACE", "0"))),
    )
    kernel.last_results = res

    acc = np.zeros((BS, D), np.float64)
    for c in range(NCORES):
        acc += res.results[c]["out"].astype(np.float64)
    acc += np.asarray(bo, np.float64)[None, :]
    return acc.reshape(B, S, D).astype(np.float32)
